# revision 1
# baseline (speedup 1.0000x reference)
"""Trainium2 Bass kernel for nn_Attention (non-local-attention block + sync BN).

Computation per batch element b (B=8, C_IN=256, C_OUT=128, N=4096):
    theta = theta_w @ x + theta_b          [128, 4096]
    phi   = phi_w @ x + phi_b              [128, 4096]
    g     = g_w @ x + g_b                  [128, 4096]
    f     = theta^T @ phi / N              [4096, 4096]  -- never materialized:
    y     = g @ f^T = (phi @ g^T)^T theta / N    (associativity; no softmax!)
    w_y   = W_w @ y  (+ W_b, cancels in BN)[256, 4096]
    out   = BN(w_y) * gamma + beta + x     (BN stats over all (B, N) -> AllGather)

KEY: with no softmax between the einsums, y = B^T @ theta / N where
B = phi @ g^T is only [128, 128] -- 32x fewer MACs than materializing the
N x N attention matrix, and the whole PSUM->SBUF f-copy pipeline vanishes.
phi^T and g^T are produced directly in [128m x 128c] blocks by swapping
the conv matmul operands (lhsT = x chunk); B accumulates one m-block
behind them in a single PSUM bank.

Sharding: data-parallel over batch across 8 NeuronCores (one element per
core); 1x1-conv weights replicated; BN batch stats synced with a tiny
[128,4] fp32 AllGather (+ local rank-sum).  Compute dtype bf16 with fp32
PSUM accumulation; the residual uses the exact fp32 x.

A tiny early dummy AllGather (result unread -- a reader would drag the
entry barrier into the pipeline) warms the collectives firmware; x32
rides the sync HWDGE ring behind the x16 chunks so its bulk transfers
cannot delay the x16 completion semaphores the first convs wait on;
BN sums ride free on the wy copy's ACT accumulator.
"""

import contextlib

import numpy as np
import ml_dtypes

import concourse.bass as bass  # noqa: F401  (registers engines)
import concourse.tile as tile
from concourse import bacc, mybir
from concourse import bass_utils

N_CORES = 8
B, C_IN, C_OUT, N = 8, 256, 128, 4096
P = 128
NCH = N // 512    # 8 column chunks of 512
MCH = N // 128    # 32 m-chunks of 128
KPAIR = MCH // 2  # 16 fT pairs per n-chunk
LAG = 3           # y-matmul lag (iterations) behind fT matmuls
BN_EPS = 1e-5

F32 = mybir.dt.float32
BF16 = mybir.dt.bfloat16
AF = mybir.ActivationFunctionType
ALU = mybir.AluOpType
AX = mybir.AxisListType


def _build_module():
    nc = bacc.Bacc("TRN2", target_bir_lowering=False, debug=False,
                   enable_asserts=True, num_devices=N_CORES)

    x32 = nc.dram_tensor("x32", [C_IN, N], F32, kind="ExternalInput").ap()
    x16 = nc.dram_tensor("x16", [C_IN, N], BF16, kind="ExternalInput").ap()
    # wpack columns: thw0 thw1 phw0 phw1 gw0 gw1 WwA WwB (8 x [128,128] bf16)
    wpack = nc.dram_tensor("wpack", [P, 1024], BF16, kind="ExternalInput").ap()
    # bpack columns: thb(1) gam(2) bet(2) gbb(128) pbb(128)
    bpack = nc.dram_tensor("bpack", [P, 261], F32, kind="ExternalInput").ap()
    out = nc.dram_tensor("out", [C_IN, N], F32, kind="ExternalOutput").ap()

    with contextlib.ExitStack() as ctx:
        tc = ctx.enter_context(tile.TileContext(nc))
        pp = ctx.enter_context(tc.tile_pool(name="persist", bufs=1))
        ysb = ctx.enter_context(tc.tile_pool(name="ysb", bufs=3))
        sqp = ctx.enter_context(tc.tile_pool(name="sqp", bufs=2))
        op = ctx.enter_context(tc.tile_pool(name="outp", bufs=3))
        ps_cv = ctx.enter_context(tc.tile_pool(name="pscv", bufs=2, space="PSUM"))
        ps_g = ctx.enter_context(tc.tile_pool(name="psg", bufs=3, space="PSUM"))
        ps_b = ctx.enter_context(tc.tile_pool(name="psb", bufs=1, space="PSUM"))
        ps_y = ctx.enter_context(tc.tile_pool(name="psy", bufs=2, space="PSUM"))
        dram = ctx.enter_context(tc.tile_pool(name="dram", bufs=1, space="DRAM"))

        # ---- persistent SBUF tensors ----
        x16h = [pp.tile([P, N], BF16, tag=f"x16_{h}", name=f"x16_{h}")
                for h in range(2)]
        x32h = [pp.tile([P, N], F32, tag=f"x32_{h}", name=f"x32_{h}")
                for h in range(2)]
        th_t = pp.tile([P, N], BF16, tag="th")
        pt_t = pp.tile([P, N], BF16, tag="pt")       # phi^T in 32 [128m x 128c] blocks
        gt_t = pp.tile([P, N], BF16, tag="gt")       # g^T in 32 [128m x 128c] blocks
        wy_t = [pp.tile([P, N], F32, tag=f"wy{h}", name=f"wy{h}") for h in range(2)]
        stat_s = pp.tile([P, 16], F32, tag="stat_s")  # per-chunk sums
        stat_q = pp.tile([P, 16], F32, tag="stat_q")  # per-chunk sum-of-squares

        wp_t = pp.tile([P, 1024], BF16, tag="wp")
        bp_t = pp.tile([P, 261], F32, tag="bp")
        eps_t = pp.tile([P, 1], F32, tag="eps")
        nc.gpsimd.memset(eps_t[:], BN_EPS)
        warm_t = pp.tile([P, 1], F32, tag="warm")

        def cs(i, w):  # column slice helper
            return slice(i * w, (i + 1) * w)

        # weight DMAs first (small), then x16 chunks so the convs start early,
        # x32 last via SWDGE (only needed for the tail residual)
        nc.scalar.dma_start(wp_t[:], wpack[:, :])
        nc.scalar.dma_start(bp_t[:], bpack[:, :])
        nc.scalar.dma_start(x16h[1][:, cs(0, 1024)], x16[P:2 * P, cs(0, 1024)])
        for q in range(4):
            nc.sync.dma_start(x16h[0][:, cs(q, 1024)], x16[0:P, cs(q, 1024)])
            if q > 0:
                nc.scalar.dma_start(x16h[1][:, cs(q, 1024)],
                                    x16[P:2 * P, cs(q, 1024)])
        thw_t = [wp_t[:, cs(k, P)] for k in range(2)]
        phw_t = [wp_t[:, cs(2 + k, P)] for k in range(2)]
        gw_t = [wp_t[:, cs(4 + k, P)] for k in range(2)]
        Ww_h = [wp_t[:, cs(6 + h, P)] for h in range(2)]
        thb_t = bp_t[:, 0:1]
        gam_t = bp_t[:, 1:3]
        bet_t = bp_t[:, 3:5]
        gbb_t = bp_t[:, 5:133]
        pbb_t = bp_t[:, 133:261]

        # dummy tiny AllGathers: warm the ncfw path, overlapped with compute,
        # so the real stats collective at the tail runs at floor cost
        in_d = dram.tile([P, 1], F32)
        out_d = dram.tile([P * N_CORES, 1], F32)
        nc.scalar.dma_start(in_d[:], eps_t[:])
        # x32 on the scalar ring AFTER all its x16 chunks: ring order means
        # every x16 completion semaphore fires before these bulk transfers
        # start competing for SDMA engines; only the tail residual needs x32
        for h in range(2):
            for q in range(4):
                nc.scalar.dma_start(x32h[h][:, cs(q, 1024)],
                                    x32[h * P:(h + 1) * P, cs(q, 1024)])
        nc.gpsimd.collective_compute(
            "AllGather", ALU.bypass,
            replica_groups=[list(range(N_CORES))],
            ins=[in_d.opt()], outs=[out_d.opt()],
        )

        # y = B^T @ theta / N with B = phi @ g^T  ([128,128] -- the N x N
        # attention matrix is never materialized; no softmax makes the two
        # einsums associative).  theta_w/theta_b carry the 1/N factor (host).

        def emit_t_conv(w_t, b_t, dst, j):       # [c', 512] conv chunk
            ps = ps_cv.tile([P, 512], F32, tag="cv", name="ps_conv")
            nc.tensor.matmul(ps[:], w_t[0], x16h[0][:, cs(j, 512)],
                             start=True, stop=False)
            nc.tensor.matmul(ps[:], w_t[1], x16h[1][:, cs(j, 512)],
                             start=False, stop=True)
            nc.scalar.activation(dst[:, cs(j, 512)], ps[:], AF.Identity,
                                 bias=b_t)

        def emit_tr_conv(w_t, bcast, dst, m):    # transposed [128m, c] block
            ps = ps_g.tile([P, P], F32, tag="g", name="ps_tr")
            nc.tensor.matmul(ps[:], x16h[0][:, cs(m, P)], w_t[0],
                             start=True, stop=False)
            nc.tensor.matmul(ps[:], x16h[1][:, cs(m, P)], w_t[1],
                             start=False, stop=True)
            nc.vector.tensor_tensor(dst[:, cs(m, P)], ps[:], bcast,
                                    op=ALU.add)

        def emit_w_block(j, y_sb):
            for h in range(2):
                w_ps = ps_cv.tile([P, 512], F32, tag="cv", name="ps_w")
                nc.tensor.matmul(w_ps[:], Ww_h[h], y_sb[:],
                                 start=True, stop=True)
                col = h * NCH + j
                nc.scalar.activation(wy_t[h][:, cs(j, 512)], w_ps[:], AF.Copy,
                                     accum_out=stat_s[:, col:col + 1])
                wyc = wy_t[h][:, cs(j, 512)]
                sq = sqp.tile([P, 512], F32, tag="sq", name="sq")
                if (j + h) % 2 == 0:
                    nc.scalar.activation(sq[:], wyc, AF.Square,
                                         accum_out=stat_q[:, col:col + 1])
                else:
                    nc.vector.scalar_tensor_tensor(
                        sq[:], wyc, 1.0, wyc, op0=ALU.mult, op1=ALU.mult,
                        accum_out=stat_q[:, col:col + 1])

        # conv phase, paced by the x16 chunk DMAs; B accumulates one m-block
        # behind the phi^T/g^T convs so the PE never waits on the DVE adds
        BLAG = 3
        b_ps = ps_b.tile([P, P], F32, tag="b", name="b_ps")
        for j in range(NCH):
            emit_t_conv(thw_t, thb_t, th_t, j)
            for m in range(4 * j, 4 * j + 4):
                emit_tr_conv(phw_t, pbb_t[:], pt_t, m)
                emit_tr_conv(gw_t, gbb_t[:], gt_t, m)
                if m >= BLAG:
                    mb = m - BLAG
                    nc.tensor.matmul(b_ps[:], pt_t[:, cs(mb, P)],
                                     gt_t[:, cs(mb, P)],
                                     start=(mb == 0), stop=False)
        nc.scalar.activation(warm_t[:], eps_t[:], AF.Sqrt)  # preload ACT table
        for mb in range(MCH - BLAG, MCH):
            nc.tensor.matmul(b_ps[:], pt_t[:, cs(mb, P)], gt_t[:, cs(mb, P)],
                             start=False, stop=(mb == MCH - 1))
        b_sb = pp.tile([P, P], BF16, tag="b_sb")
        nc.vector.tensor_copy(b_sb[:], b_ps[:])

        # y = B^T theta, then W projection + BN stats, W lagging one chunk
        y_sbs = []
        for j in range(NCH):
            y_ps = ps_y.tile([P, 512], F32, tag="y", name="y_ps")
            nc.tensor.matmul(y_ps[:], b_sb[:], th_t[:, cs(j, 512)],
                             start=True, stop=True)
            y_sb = ysb.tile([P, 512], BF16, tag="y_sb", name="y_sb")
            nc.vector.tensor_copy(y_sb[:], y_ps[:])
            y_sbs.append(y_sb)
            if j >= 2:
                emit_w_block(j - 2, y_sbs[j - 2])
        emit_w_block(NCH - 2, y_sbs[NCH - 2])
        emit_w_block(NCH - 1, y_sbs[NCH - 1])

        # ---- BN stats: local reduce, AllReduce, affine params ----
        s4 = pp.tile([P, 4], F32, tag="s4")
        nc.vector.reduce_sum(s4[:, 0:1], stat_s[:, 0:NCH], axis=AX.X)
        nc.vector.reduce_sum(s4[:, 1:2], stat_s[:, NCH:2 * NCH], axis=AX.X)
        nc.vector.reduce_sum(s4[:, 2:3], stat_q[:, 0:NCH], axis=AX.X)
        nc.vector.reduce_sum(s4[:, 3:4], stat_q[:, NCH:2 * NCH], axis=AX.X)
        del stat_s, stat_q
        in_b = dram.tile([P, 4], F32)
        out_b = dram.tile([P * N_CORES, 4], F32)
        nc.sync.dma_start(in_b[:], s4[:])
        nc.gpsimd.collective_compute(
            "AllGather", ALU.bypass,
            replica_groups=[list(range(N_CORES))],
            ins=[in_b.opt()], outs=[out_b.opt()],
        )
        g32 = pp.tile([P, 32], F32, tag="g32")
        nc.gpsimd.dma_start(g32[:].rearrange("p (c r) -> p c r", r=N_CORES),
                            out_b[:].rearrange("(r p) c -> p c r", p=P))
        g4 = pp.tile([P, 4], F32, tag="g4")
        nc.vector.reduce_sum(g4[:], g32[:].rearrange("p (c r) -> p c r",
                                                     r=N_CORES), axis=AX.X)

        inv_cnt = 1.0 / (B * N)
        m4 = pp.tile([P, 4], F32, tag="m4")      # [mn(2) | ms(2)]
        var = pp.tile([P, 2], F32, tag="var")
        tmp = pp.tile([P, 2], F32, tag="tmp")
        sd = pp.tile([P, 2], F32, tag="sd")
        rstd = pp.tile([P, 2], F32, tag="rstd")
        scl = pp.tile([P, 2], F32, tag="scl")
        bia = pp.tile([P, 2], F32, tag="bia")
        nc.vector.tensor_scalar_mul(m4[:], g4[:], inv_cnt)
        mn = m4[:, 0:2]
        nc.vector.tensor_mul(tmp[:], mn, mn)
        nc.vector.tensor_sub(var[:], m4[:, 2:4], tmp[:])
        nc.scalar.activation(sd[:], var[:], AF.Sqrt, bias=eps_t[:, 0:1])
        nc.vector.reciprocal(rstd[:], sd[:])
        nc.vector.tensor_mul(scl[:], rstd[:], gam_t)
        nc.vector.tensor_mul(tmp[:], mn, scl[:])
        nc.vector.tensor_sub(bia[:], bet_t, tmp[:])

        # ---- normalize + residual + store ----
        for idx in range(NCH):
            h, j = divmod(idx, NCH // 2)
            o1 = op.tile([P, 1024], F32, tag="o1", name="o1")
            o2 = op.tile([P, 1024], F32, tag="o2", name="o2")
            nc.scalar.activation(o1[:], wy_t[h][:, cs(j, 1024)],
                                 AF.Identity, bias=bia[:, h:h + 1],
                                 scale=scl[:, h:h + 1])
            nc.vector.tensor_add(o2[:], o1[:], x32h[h][:, cs(j, 1024)])
            deng = nc.sync if idx % 2 == 0 else nc.scalar
            deng.dma_start(out[h * P:(h + 1) * P, cs(j, 1024)], o2[:])

    nc.compile()
    return nc


_CACHE = {}


def _get_module():
    if "nc" not in _CACHE:
        _CACHE["nc"] = _build_module()
    return _CACHE["nc"]


def _prep_in_maps(x, g_w, g_b, theta_w, theta_b, phi_w, phi_b, W_w, W_b,
                  bn_gamma, bn_beta):
    bf = ml_dtypes.bfloat16
    f32 = np.float32
    x = np.ascontiguousarray(x, dtype=f32)
    thwT = (theta_w.T / N).astype(bf)
    phwT = phi_w.T.astype(bf)
    gwT = g_w.T.astype(bf)
    WwT = W_w.T.astype(bf)
    wpack = np.concatenate(
        [thwT[0:P], thwT[P:2 * P], phwT[0:P], phwT[P:2 * P],
         gwT[0:P], gwT[P:2 * P], WwT[:, 0:P], WwT[:, P:2 * P]], axis=1)
    bpack = np.concatenate(
        [(theta_b / N).reshape(P, 1).astype(f32),
         bn_gamma.reshape(2, P).T.astype(f32),
         bn_beta.reshape(2, P).T.astype(f32),
         np.broadcast_to(g_b[None, :].astype(f32), (P, C_OUT)),
         np.broadcast_to(phi_b[None, :].astype(f32), (P, C_OUT))], axis=1)
    shared = {
        "wpack": np.ascontiguousarray(wpack),
        "bpack": np.ascontiguousarray(bpack),
    }
    in_maps = []
    for i in range(N_CORES):
        m = dict(shared)
        m["x32"] = x[i]
        m["x16"] = np.ascontiguousarray(x[i].astype(bf))
        in_maps.append(m)
    return in_maps


def _run(inputs, trace=False, trace_cores=None):
    nc = _get_module()
    in_maps = _prep_in_maps(**inputs)
    res = bass_utils.run_bass_kernel_spmd(
        nc, in_maps, core_ids=list(range(N_CORES)),
        trace=trace, trace_cores=trace_cores,
    )
    out = np.stack([res.results[i]["out"] for i in range(N_CORES)], axis=0)
    return out.astype(np.float32), res


def kernel(**inputs) -> np.ndarray:
    out, _ = _run(inputs, trace=False)
    return out



# revision 16
# speedup vs baseline: 1.0956x; 1.0956x over previous
"""Trainium2 Bass kernel for nn_Attention (non-local-attention block + sync BN).

Computation per batch element b (B=8, C_IN=256, C_OUT=128, N=4096):
    theta = theta_w @ x (+ theta_b)        [128, 4096]
    phi   = phi_w @ x + phi_b              [128, 4096]
    g     = g_w @ x + g_b                  [128, 4096]
    f     = theta^T @ phi / N              [4096, 4096]  (no softmax!)
    y     = g @ f^T;  w_y = W_w @ y (+W_b) [256, 4096]
    out   = BN(w_y) * gamma + beta + x     (BN stats over all (B, N))

KEY ALGEBRA: with no softmax everything after the x-dependent convs is
LINEAR, so the entire theta/y/W chain collapses into one [256,256] matrix:
    Bt  = g @ phi^T                         [128, 128]   (cross-correlation)
    A^T = (theta_w/N)^T @ Bt^T @ W_w^T      [256, 256]
    w_y = A @ x         (theta_b and W_b shift every column equally ->
                         they cancel inside BN and are dropped entirely)
So only TWO convs (phi, g) touch [*, 4096] data, the theta conv and the
separate W-projection pass vanish, and w_y = A @ x streams straight from
the resident bf16 x.  The residual uses bf16 x as well (~1e-3 rel err).

Sharding: data-parallel over batch, one element per core; BN stats synced
with a single [128,4] fp32 AllReduce.  The collectives firmware pays a
one-time ~20us init that starts at the FIRST doorbell and ends only after
ALL ranks ring: a tiny input-independent dummy AllGather is therefore rung
at t~1us on every core (its input is a 512B DMA that leads the sync ring)
so the init + rendezvous runs entirely under the compute phase, and the
real AllReduce only pays its ~7us op latency.  AllReduce (not AllGather)
returns pre-summed stats, killing the gather DMA + gpsimd drain that the
gather variant costs after the collective.
"""

import contextlib

import numpy as np
import ml_dtypes

import concourse.bass as bass  # noqa: F401  (registers engines)
import concourse.tile as tile
from concourse import bacc, mybir
from concourse import bass_utils

N_CORES = 8
B, C_IN, C_OUT, N = 8, 256, 128, 4096
P = 128
MCH = N // P      # 32 column blocks of 128
BLAG = 3          # Bt accumulation lag (blocks) behind the conv copies
BN_EPS = 1e-5

F32 = mybir.dt.float32
BF16 = mybir.dt.bfloat16
AF = mybir.ActivationFunctionType
ALU = mybir.AluOpType
AX = mybir.AxisListType


def _build_module():
    nc = bacc.Bacc("TRN2", target_bir_lowering=False, debug=False,
                   enable_asserts=True, num_devices=N_CORES)

    x16 = nc.dram_tensor("x16", [C_IN, N], BF16, kind="ExternalInput").ap()
    # wpack cols: phwT(2x128) gwT(2x128) thwN(256) WwT(256) pbb4(512) gbb4(512)
    wpack = nc.dram_tensor("wpack", [P, 2048], BF16, kind="ExternalInput").ap()
    # bpack cols: gam(2) bet(2) eps(1)
    bpack = nc.dram_tensor("bpack", [P, 5], F32, kind="ExternalInput").ap()
    out = nc.dram_tensor("out", [C_IN, N], F32, kind="ExternalOutput").ap()

    with contextlib.ExitStack() as ctx:
        tc = ctx.enter_context(tile.TileContext(nc))
        pp = ctx.enter_context(tc.tile_pool(name="persist", bufs=1))
        sqp = ctx.enter_context(tc.tile_pool(name="sqp", bufs=2))
        op = ctx.enter_context(tc.tile_pool(name="outp", bufs=3))
        ps_cv = ctx.enter_context(tc.tile_pool(name="pscv", bufs=2, space="PSUM"))
        ps_b = ctx.enter_context(tc.tile_pool(name="psb", bufs=1, space="PSUM"))
        ps_s = ctx.enter_context(tc.tile_pool(name="pss", bufs=1, space="PSUM"))
        ps_y = ctx.enter_context(tc.tile_pool(name="psy", bufs=4, space="PSUM"))
        dram = ctx.enter_context(tc.tile_pool(name="dram", bufs=1, space="DRAM"))

        def cs(i, w):  # column slice helper
            return slice(i * w, (i + 1) * w)

        # ---- persistent SBUF ----
        x16h = [pp.tile([P, N], BF16, tag=f"x16_{h}", name=f"x16_{h}")
                for h in range(2)]
        pt_t = pp.tile([P, N], BF16, tag="pt")       # phi^T in [128n x 128c] blocks
        gt_t = pp.tile([P, N], BF16, tag="gt")       # g^T   in [128n x 128c] blocks
        wy_t = [pp.tile([P, N], F32, tag=f"wy{h}", name=f"wy{h}") for h in range(2)]
        wp_t = pp.tile([P, 2048], BF16, tag="wp")
        bp_t = pp.tile([P, 5], F32, tag="bp")
        stat_s = pp.tile([P, 16], F32, tag="stat_s")
        stat_q = pp.tile([P, 16], F32, tag="stat_q")
        warm_t = pp.tile([P, 1], F32, tag="warm")

        in_d = dram.tile([P, 1], F32)
        out_d = dram.tile([P * N_CORES, 1], F32)
        in_b = dram.tile([P, 4], F32)
        out_b = dram.tile([P, 4], F32)

        # ---- dummy collective rung ASAP: input is a 512B DMA that leads the
        # sync ring, and the collective is the first gpsimd instruction, so
        # every rank's doorbell rings ~1us in and the firmware init +
        # all-rank rendezvous runs under the compute phase.
        nc.sync.dma_start(in_d[:], bpack[:, 4:5])
        nc.gpsimd.collective_compute(
            "AllGather", ALU.bypass,
            replica_groups=[list(range(N_CORES))],
            ins=[in_d.opt()], outs=[out_d.opt()],
        )

        # ---- input DMAs: weights lead the scalar ring; x halves split over
        # both rings so block m of both halves lands early and together.
        nc.scalar.dma_start(wp_t[:], wpack[:, :])
        nc.scalar.dma_start(bp_t[:], bpack[:, :])
        for q in range(8):
            nc.sync.dma_start(x16h[0][:, cs(q, 512)], x16[0:P, cs(q, 512)])
            nc.scalar.dma_start(x16h[1][:, cs(q, 512)], x16[P:2 * P, cs(q, 512)])

        phw = [wp_t[:, cs(k, P)] for k in range(2)]
        gw = [wp_t[:, cs(2 + k, P)] for k in range(2)]
        thw = wp_t[:, 512:768]       # theta_w/N as [Co, Ci]
        WwT = wp_t[:, 768:1024]      # W_w^T as [Co, Ci]
        pbb4 = wp_t[:, 1024:1536]    # phi_b broadcast, 4 blocks [128, 512]
        gbb4 = wp_t[:, 1536:2048]    # g_b broadcast, 4 blocks [128, 512]
        gam2 = bp_t[:, 0:2]
        bet2 = bp_t[:, 2:4]
        eps1 = bp_t[:, 4:5]

        # preload ACT tables (Identity for the tail, Sqrt for BN) off the
        # critical path
        nc.scalar.activation(warm_t[:], eps1, AF.Identity, bias=eps1)
        nc.scalar.activation(warm_t[:], eps1, AF.Sqrt, bias=eps1)

        # ---- phase 1: transposed convs (lhsT = x block shared by phi and g
        # matmuls), 4 blocks per PSUM bank, one wide bias-add copy per bank;
        # Bt = g @ phi^T accumulates one chunk behind the copies
        bT_ps = ps_b.tile([P, P], F32, tag="bT", name="bT_ps")

        for q in range(8):
            php = ps_cv.tile([P, 512], F32, tag="cv", name="php")
            gps = ps_cv.tile([P, 512], F32, tag="cv", name="gps")
            for b in range(4):
                m = 4 * q + b
                xb0 = x16h[0][:, cs(m, P)]
                xb1 = x16h[1][:, cs(m, P)]
                c = cs(b, P)
                nc.tensor.matmul(php[:, c], xb0, phw[0], start=True, stop=False)
                nc.tensor.matmul(gps[:, c], xb0, gw[0], start=True, stop=False)
                nc.tensor.matmul(php[:, c], xb1, phw[1], start=False, stop=True)
                nc.tensor.matmul(gps[:, c], xb1, gw[1], start=False, stop=True)
            nc.vector.tensor_tensor(pt_t[:, cs(q, 512)], php[:], pbb4,
                                    op=ALU.add)
            nc.vector.tensor_tensor(gt_t[:, cs(q, 512)], gps[:], gbb4,
                                    op=ALU.add)
            if q >= 1:
                for mb in range(4 * (q - 1), 4 * q):
                    nc.tensor.matmul(bT_ps[:], gt_t[:, cs(mb, P)],
                                     pt_t[:, cs(mb, P)],
                                     start=(mb == 0), stop=False)
        for mb in range(MCH - 4, MCH):
            nc.tensor.matmul(bT_ps[:], gt_t[:, cs(mb, P)], pt_t[:, cs(mb, P)],
                             start=False, stop=(mb == MCH - 1))

        # ---- phase 2: A^T = (theta_w/N)^T Bt^T W_w^T via two tiny matmuls
        bT_sb = pp.tile([P, P], BF16, tag="bT_sb")
        nc.vector.tensor_copy(bT_sb[:], bT_ps[:])
        m1_ps = ps_s.tile([P, 2 * P], F32, tag="p2", name="m1_ps")
        nc.tensor.matmul(m1_ps[:], bT_sb[:], WwT, start=True, stop=True)
        m1_sb = pp.tile([P, 2 * P], BF16, tag="m1_sb")
        nc.scalar.activation(m1_sb[:], m1_ps[:], AF.Copy)
        at_sb = [pp.tile([P, 2 * P], BF16, tag=f"at{h}", name=f"at{h}")
                 for h in range(2)]
        for h in range(2):
            at_ps = ps_s.tile([P, 2 * P], F32, tag="p2", name="at_ps")
            nc.tensor.matmul(at_ps[:], thw[:, cs(h, P)], m1_sb[:],
                             start=True, stop=True)
            if h == 0:
                nc.vector.tensor_copy(at_sb[h][:], at_ps[:])
            else:
                nc.scalar.activation(at_sb[h][:], at_ps[:], AF.Copy)

        # ---- phase 3: w_y = A @ x in [128,512] chunks + BN stat accums.
        # lhsT reuse: 4 chunks per LDWEIGHTS by sweeping kh inside the group.
        for h in range(2):
            for qg in range(2):
                tiles = []
                for qi in range(4):
                    tiles.append(ps_y.tile([P, 512], F32, tag="y",
                                           name="wy_ps"))
                for kh in range(2):
                    for qi in range(4):
                        q = qg * 4 + qi
                        nc.tensor.matmul(tiles[qi][:],
                                         at_sb[kh][:, cs(h, P)],
                                         x16h[kh][:, cs(q, 512)],
                                         start=(kh == 0), stop=(kh == 1))
                for qi in range(4):
                    q = qg * 4 + qi
                    col = h * 8 + q
                    nc.scalar.activation(wy_t[h][:, cs(q, 512)], tiles[qi][:],
                                         AF.Copy,
                                         accum_out=stat_s[:, col:col + 1])
                    wyc = wy_t[h][:, cs(q, 512)]
                    sq = sqp.tile([P, 512], BF16, tag="sq", name="sq")
                    nc.vector.scalar_tensor_tensor(
                        sq[:], wyc, 1.0, wyc,
                        op0=ALU.mult, op1=ALU.mult,
                        accum_out=stat_q[:, col:col + 1])

        # ---- BN stats: local reduce, AllReduce, affine params ----
        s4 = pp.tile([P, 4], F32, tag="s4")
        nc.vector.reduce_sum(s4[:, 0:1], stat_s[:, 0:8], axis=AX.X)
        nc.vector.reduce_sum(s4[:, 1:2], stat_s[:, 8:16], axis=AX.X)
        nc.vector.reduce_sum(s4[:, 2:3], stat_q[:, 0:8], axis=AX.X)
        nc.vector.reduce_sum(s4[:, 3:4], stat_q[:, 8:16], axis=AX.X)
        nc.sync.dma_start(in_b[:], s4[:])
        nc.gpsimd.collective_compute(
            "AllReduce", ALU.add,
            replica_groups=[list(range(N_CORES))],
            ins=[in_b.opt()], outs=[out_b.opt()],
        )
        s4g = pp.tile([P, 4], F32, tag="s4g")
        nc.sync.dma_start(s4g[:], out_b[:])

        inv_cnt = 1.0 / (B * N)
        m4 = pp.tile([P, 4], F32, tag="m4")
        var = pp.tile([P, 2], F32, tag="var")
        tmp = pp.tile([P, 2], F32, tag="tmp")
        sd = pp.tile([P, 2], F32, tag="sd")
        rstd = pp.tile([P, 2], F32, tag="rstd")
        scl = pp.tile([P, 2], F32, tag="scl")
        bia = pp.tile([P, 2], F32, tag="bia")
        nc.vector.tensor_scalar_mul(m4[:], s4g[:], inv_cnt)
        mn = m4[:, 0:2]
        nc.vector.tensor_mul(tmp[:], mn, mn)
        nc.vector.tensor_sub(var[:], m4[:, 2:4], tmp[:])
        nc.scalar.activation(sd[:], var[:], AF.Sqrt, bias=eps1)
        nc.vector.reciprocal(rstd[:], sd[:])
        nc.vector.tensor_mul(scl[:], rstd[:], gam2)
        nc.vector.tensor_mul(tmp[:], mn, scl[:])
        nc.vector.tensor_sub(bia[:], bet2, tmp[:])

        # ---- tail: normalize + bf16 residual + store, 3-way engine split
        for idx in range(8):
            h, jj = divmod(idx, 4)
            o1 = op.tile([P, 1024], F32, tag="o1", name="o1")
            o2 = op.tile([P, 1024], F32, tag="o2", name="o2")
            nc.scalar.activation(o1[:], wy_t[h][:, cs(jj, 1024)],
                                 AF.Identity, bias=bia[:, h:h + 1],
                                 scale=scl[:, h:h + 1])
            adde = nc.gpsimd if idx in (2, 5) else nc.vector
            adde.tensor_tensor(o2[:], o1[:], x16h[h][:, cs(jj, 1024)],
                               op=ALU.add)
            deng = nc.sync if idx % 2 == 0 else nc.scalar
            deng.dma_start(out[h * P:(h + 1) * P, cs(jj, 1024)], o2[:])

    nc.compile()
    return nc


_CACHE = {}


def _get_module():
    if "nc" not in _CACHE:
        _CACHE["nc"] = _build_module()
    return _CACHE["nc"]


def _prep_in_maps(x, g_w, g_b, theta_w, theta_b, phi_w, phi_b, W_w, W_b,
                  bn_gamma, bn_beta):
    bf = ml_dtypes.bfloat16
    f32 = np.float32
    x = np.ascontiguousarray(x, dtype=f32)
    phwT = phi_w.T.astype(bf)                      # [256, 128]
    gwT = g_w.T.astype(bf)
    thwN = (theta_w / N).astype(bf)                # [128, 256]
    WwT = W_w.T.astype(bf)                         # [128, 256]
    pbb4 = np.tile(np.broadcast_to(phi_b[None, :].astype(bf), (P, C_OUT)),
                   (1, 4))
    gbb4 = np.tile(np.broadcast_to(g_b[None, :].astype(bf), (P, C_OUT)),
                   (1, 4))
    wpack = np.concatenate(
        [phwT[0:P], phwT[P:2 * P], gwT[0:P], gwT[P:2 * P],
         thwN, WwT, pbb4, gbb4], axis=1)
    bpack = np.concatenate(
        [bn_gamma.reshape(2, P).T.astype(f32),
         bn_beta.reshape(2, P).T.astype(f32),
         np.full((P, 1), BN_EPS, dtype=f32)], axis=1)
    shared = {
        "wpack": np.ascontiguousarray(wpack),
        "bpack": np.ascontiguousarray(bpack),
    }
    in_maps = []
    for i in range(N_CORES):
        m = dict(shared)
        m["x16"] = np.ascontiguousarray(x[i].astype(bf))
        in_maps.append(m)
    return in_maps


def _run(inputs, trace=False, trace_cores=None):
    nc = _get_module()
    in_maps = _prep_in_maps(**inputs)
    res = bass_utils.run_bass_kernel_spmd(
        nc, in_maps, core_ids=list(range(N_CORES)),
        trace=trace, trace_cores=trace_cores,
    )
    out = np.stack([res.results[i]["out"] for i in range(N_CORES)], axis=0)
    return out.astype(np.float32), res


def kernel(**inputs) -> np.ndarray:
    out, _ = _run(inputs, trace=False)
    return out
